# revision 15
# baseline (speedup 1.0000x reference)
"""Trainium2 Bass kernel for the ESM contrastive projection head loss.

Problem (hardcoded): x [512, 512, 960] f32; two 2-layer MLPs (codon for batch
rows 0:256, amino for 256:512) applied to mean-pooled x; pairwise cosine
similarity of the concatenated projections z [512, 240]; diag-masked,
temperature-scaled InfoNCE-style NLL, mean over rows.

Strategy: data-parallel over batch across 8 NeuronCores (64 rows each).
Each core streams its 126 MB x-shard near the HBM roofline (x-slab DMAs
alternate between the two HWDGE rings), mean-pools via a DVE add-tree plus
fp16 window matmuls that accumulate pooled^T directly in PSUM.  The 64 rows
are processed in 4 pipelined chunks of 16: each chunk's MLP (fp16 weights),
row-normalize (z_hat = z/|z|), AllGather of z_hat [16,240], and post-gather
transposes all overlap the streaming of later chunks; only the last chunk's
back-end is on the critical tail.  The NxN similarity is two fp16 matmuls of
pre-normalized vectors (no norm outer product); the diagonal mask folds into
one additive mask; exp uses a fixed shift (logits <= 10) so no row max is
needed; ln(row_sum) runs on the host.  Output per core: [64, 2] = (esum, pos).
"""
import contextlib
import ctypes
import os
import sys
import types

import numpy as np

B = 512
S = 512
D = 960
NCORES = 8
BPC = B // NCORES           # 64 batch rows per core
SLAB_B = 2                  # batch rows per DMA slab
NSLAB = BPC // SLAB_B       # 32
NCHUNK = 4
SPC = NSLAB // NCHUNK       # 8 slabs per chunk
RPC = BPC // NCHUNK         # 16 rows per chunk
INV_T = 10.0                # 1 / temperature
NEG_T = -655040.0           # NEG_INF / temperature
SHIFT = 10.0                # fixed logsumexp shift (logits <= ~10)
D1 = D // 2                 # 480
D2 = D // 4                 # 240

_CACHE = {}
LAST_RESULT = None
TRACE_CORES = [0]


def _install_ntff_hook():
    """Make run_bass_kernel_spmd(trace=True) work under axon (test.py only)."""
    if "antenv.axon_hooks" in sys.modules:
        return
    so_path = "/opt/axon/libaxon_pjrt.so"
    try:
        lib = ctypes.CDLL(so_path)
    except OSError:
        return
    if not hasattr(lib, "axon_start_nrt_profile"):
        return
    lib.axon_start_nrt_profile.argtypes = [ctypes.POINTER(ctypes.c_int64), ctypes.c_size_t]
    lib.axon_start_nrt_profile.restype = ctypes.c_int64
    lib.axon_stop_nrt_profile.argtypes = [ctypes.c_char_p]
    lib.axon_stop_nrt_profile.restype = ctypes.c_int64

    @contextlib.contextmanager
    def _hook(output_dir, device_ids):
        import jax
        jax.devices()
        if device_ids:
            ids = (ctypes.c_int64 * len(device_ids))(*device_ids)
            rc = lib.axon_start_nrt_profile(ids, len(device_ids))
        else:
            rc = lib.axon_start_nrt_profile(None, 0)
        if rc != 0:
            raise RuntimeError(f"axon_start_nrt_profile rc={rc}")
        try:
            yield
        finally:
            n = lib.axon_stop_nrt_profile(str(output_dir).encode())
            print(f"profile: {n} file(s) written to {output_dir}", file=sys.stderr)

    mod = types.ModuleType("antenv.axon_hooks")
    mod.get_axon_ntff_profile_hook = lambda: _hook
    mod.set_axon_ntff_profile_hook = lambda h: None
    sys.modules["antenv.axon_hooks"] = mod


def _build_nc():
    import concourse.tile as tile
    from concourse import bacc, mybir

    f32 = mybir.dt.float32
    f16 = mybir.dt.float16
    add = mybir.AluOpType.add
    mult = mybir.AluOpType.mult
    amax = mybir.AluOpType.max
    AF = mybir.ActivationFunctionType
    AX = mybir.AxisListType

    nc = bacc.Bacc("TRN2", target_bir_lowering=False, debug=False,
                   enable_asserts=True, num_devices=NCORES)

    xs = nc.dram_tensor("xs", [BPC, S, D], f32, kind="ExternalInput").ap()
    w1h = nc.dram_tensor("w1h", [128, 8, 4, 128], f16, kind="ExternalInput").ap()
    b1h = nc.dram_tensor("b1h", [120, 4], f32, kind="ExternalInput").ap()
    w2h = nc.dram_tensor("w2h", [120, 4, D2], f16, kind="ExternalInput").ap()
    b2r = nc.dram_tensor("b2r", [1, D2], f16, kind="ExternalInput").ap()
    jwh = nc.dram_tensor("jwh", [128, SPC, RPC], f16, kind="ExternalInput").ap()
    ident = nc.dram_tensor("ident", [128, 128], f32, kind="ExternalInput").ap()
    addm = nc.dram_tensor("addm", [BPC, B], f32, kind="ExternalInput").ap()
    posm = nc.dram_tensor("posm", [BPC, B], f32, kind="ExternalInput").ap()
    out = nc.dram_tensor("outv", [BPC, 2], f32, kind="ExternalOutput").ap()

    # K-chunk sizes for the 960-feature contraction: 7x128 + 64
    KCH = [128] * 7 + [64]

    with tile.TileContext(nc) as tc:
        with contextlib.ExitStack() as ctx:
            ep = ctx.enter_context
            consts = ep(tc.tile_pool(name="consts", bufs=1))
            xpool = ep(tc.tile_pool(name="xslab", bufs=3))
            t1pool = ep(tc.tile_pool(name="t1", bufs=2))
            t2pool = ep(tc.tile_pool(name="t2", bufs=2))
            apool = ep(tc.tile_pool(name="acc", bufs=2))
            lastp = ep(tc.tile_pool(name="lastp", bufs=1))
            spool = ep(tc.tile_pool(name="small", bufs=1))
            hpool = ep(tc.tile_pool(name="hbuf", bufs=2))
            zhpool = ep(tc.tile_pool(name="zhat", bufs=2))
            zfpool = ep(tc.tile_pool(name="zfrow", bufs=2))
            dram = ep(tc.tile_pool(name="dram", bufs=1, space="DRAM"))
            ppool = ep(tc.tile_pool(name="ppool", bufs=1, space="PSUM"))
            psmm = ep(tc.tile_pool(name="psmm", bufs=1, space="PSUM"))
            psz = ep(tc.tile_pool(name="psz", bufs=1, space="PSUM"))
            pstr = ep(tc.tile_pool(name="pstr", bufs=1, space="PSUM"))
            pssim = ep(tc.tile_pool(name="pssim", bufs=1, space="PSUM"))

            # --- constants (gpsimd/SWDGE ring; x-slabs own the two HWDGE rings)
            w1_sb = consts.tile([128, 8, 4, 128], f16, tag="w1")
            nc.gpsimd.dma_start(w1_sb[:], w1h)
            w2_sb = consts.tile([120, 4, D2], f16, tag="w2")
            nc.gpsimd.dma_start(w2_sb[:], w2h)
            b1_sb = consts.tile([120, 4], f32, tag="b1")
            nc.gpsimd.dma_start(b1_sb[:], b1h)
            b2_sb = consts.tile([1, D2], f16, tag="b2")
            nc.gpsimd.dma_start(b2_sb[:], b2r)
            jw_sb = consts.tile([128, SPC, RPC], f16, tag="jw")
            nc.gpsimd.dma_start(jw_sb[:], jwh)
            ident_sb = consts.tile([128, 128], f32, tag="ident")
            nc.gpsimd.dma_start(ident_sb[:], ident)
            addm_sb = consts.tile([BPC, B], f32, tag="addm")
            nc.gpsimd.dma_start(addm_sb[:], addm)
            posm_sb = consts.tile([BPC, B], f32, tag="posm")
            nc.gpsimd.dma_start(posm_sb[:], posm)

            ones16 = consts.tile([1, RPC], f16, tag="ones16")
            nc.vector.memset(ones16[:], 1.0)
            negsh = consts.tile([BPC, 1], f32, tag="negsh")
            nc.vector.memset(negsh[:], -SHIFT)
            zeros_sb = consts.tile([120, RPC], f32, tag="zeros")
            nc.vector.memset(zeros_sb[:], 0.0)

            # --- warm up the collective path early (junk payload, folded into
            # the output so it is not dead code) ---
            wjunk = spool.tile([RPC, D2], f32, tag="wjunk")
            nc.vector.memset(wjunk[:], 0.0)
            wb = dram.tile([RPC, D2], f32, tag="wb")
            wg = dram.tile([RPC * NCORES, D2], f32, tag="wg")
            nc.gpsimd.dma_start(wb[:], wjunk[:])
            nc.gpsimd.collective_compute(
                "AllGather", mybir.AluOpType.bypass,
                replica_groups=[list(range(NCORES))],
                ins=[wb.opt()], outs=[wg.opt()],
            )
            wg_sb = spool.tile([BPC, 1], f32, tag="wg")
            nc.gpsimd.dma_start(wg_sb[:], wg[0:BPC, 0:1])

            # persistent accumulators across chunks
            zfT_sb = spool.tile([120, 2, B], f16, tag="zfT")
            zTo_sb = spool.tile([120, 2, BPC], f16, tag="zTo")

            # per-chunk DRAM collective buffers
            zb = [dram.tile([RPC, D2], f32, tag=f"zb{c}", name=f"zb{c}")
                  for c in range(NCHUNK)]
            zg = [dram.tile([RPC * NCORES, D2], f32, tag=f"zg{c}", name=f"zg{c}")
                  for c in range(NCHUNK)]

            # all 4 chunks' poolT accumulators packed into one PSUM bank
            ptall = ppool.tile([128, NCHUNK, SPC, RPC], f32, tag="ptall")
            hpall = psmm.tile([128, 4, RPC], f32, tag="hpall")
            pt = [None] * NCHUNK      # poolT PSUM slices per chunk
            back_emitted = [False] * NCHUNK

            def emit_slab(ch, i):
                """Stream slab i (2 batch rows) of chunk ch, reduce over seq,
                and accumulate pooled^T into the chunk's PSUM tile."""
                g = ch * SPC + i
                ring = nc.sync if (g % 2 == 0) else nc.scalar
                last = (ch == NCHUNK - 1 and i == SPC - 1)
                src = xs[SLAB_B * g:SLAB_B * (g + 1)].rearrange(
                    "b (q m) d -> (b q) m d", m=8)
                acc = apool.tile([128, D], f16, tag="acc")
                if not last:
                    slab = xpool.tile([128, 8, D], f32, tag="slab")
                    ring.dma_start(slab[:], src)
                    t1 = t1pool.tile([128, 4, D], f16, tag="t1")
                    nc.vector.tensor_tensor(t1[:], slab[:, 0:4, :],
                                            slab[:, 4:8, :], add)
                    t2 = t2pool.tile([128, 2, D], f16, tag="t2")
                    nc.vector.tensor_tensor(t2[:], t1[:, 0:2, :],
                                            t1[:, 2:4, :], add)
                    nc.vector.tensor_tensor(acc[:], t2[:, 0, :], t2[:, 1, :], add)
                else:
                    # final slab: 8 sub-DMAs with chained adds so the DVE tree
                    # overlaps the DMA and the serial tail is ~1 add
                    subs = [lastp.tile([128, D], f32, tag=f"sub{k}",
                                       name=f"sub{k}")
                            for k in range(8)]
                    run = lastp.tile([128, D], f32, tag="runl")
                    for k in range(8):
                        ring = nc.sync if (k % 2 == 0) else nc.scalar
                        ring.dma_start(subs[k][:], src[:, k, :])
                        if k == 1:
                            nc.vector.tensor_tensor(run[:], subs[0][:],
                                                    subs[1][:], add)
                        elif k >= 2:
                            dst = acc if k == 7 else run
                            nc.vector.tensor_tensor(dst[:], run[:],
                                                    subs[k][:], add)
                if i == 0:
                    pt[ch] = ptall[:, ch]
                for g8 in range(8):
                    K = KCH[g8]
                    # one accumulation group per chunk: start=True clears the
                    # has-written bits of the WHOLE bank, so only the first
                    # matmul of the chunk may carry it
                    nc.tensor.matmul(
                        pt[ch][0:K, g8, :],
                        acc[:, 128 * g8:128 * g8 + K],
                        jw_sb[:, i, :],
                        start=(i == 0 and g8 == 0),
                        stop=(i == SPC - 1 and g8 == 7),
                        skip_group_check=True)

            def emit_backend(ch):
                """MLP + normalize + payload DMA + AllGather for chunk ch."""
                back_emitted[ch] = True
                pT = spool.tile([128, SPC, RPC], f16, tag=f"pT{ch}")
                nc.vector.tensor_copy(pT[:], pt[ch][:])
                # MLP layer 1: h [120, 4, RPC] = relu(W1^T pooled^T + b1)
                h_sb = hpool.tile([120, 4, RPC], f16, tag="h")
                for jg in range(4):
                    hp = hpall[:, jg]
                    for kc in range(8):
                        K = KCH[kc]
                        nc.tensor.matmul(hp[:], w1_sb[0:K, kc, jg, :],
                                         pT[0:K, kc, :],
                                         start=(kc == 0), stop=(kc == 7))
                    nc.vector.scalar_tensor_tensor(
                        h_sb[:, jg, :], hp[0:120, :], b1_sb[:, jg:jg + 1],
                        zeros_sb[:], add, amax)
                # MLP layer 2 (row-major): z [RPC, 240] = h^T W2 + b2
                zp = psz.tile([RPC, D2], f32, tag="zp")
                for kc in range(4):
                    nc.tensor.matmul(zp[:], h_sb[:, kc, :], w2_sb[:, kc, :],
                                     start=(kc == 0), stop=False)
                nc.tensor.matmul(zp[:], ones16[:], b2_sb[:],
                                 start=False, stop=True)
                # normalize rows: zhat = z / |z|  (Square+accum gives the
                # row sum-of-squares in one ACT op)
                sq = zhpool.tile([RPC, D2], f32, tag="sq")
                nsq = spool.tile([RPC, 1], f32, tag=f"nsq{ch}")
                nc.scalar.activation(sq[:], zp[:], AF.Square,
                                     accum_out=nsq[:])
                nst = spool.tile([RPC, 1], f32, tag=f"nst{ch}")
                nc.scalar.sqrt(nst[:], nsq[:])
                nlr = spool.tile([RPC, 1], f32, tag=f"nlr{ch}")
                nc.vector.reciprocal(nlr[:], nst[:])
                zhat = zhpool.tile([RPC, D2], f32, tag="zhat")
                nc.vector.tensor_scalar_mul(zhat[:], zp[:], nlr[:])
                # own z^T blocks (scaled by 1/T) via PE transpose of zhat
                for og in range(2):
                    trp = pstr.tile([120, RPC], f32, tag="trp")
                    nc.tensor.transpose(trp[:], zhat[:, 120 * og:120 * (og + 1)],
                                        ident_sb[0:RPC, 0:RPC])
                    nc.vector.tensor_scalar_mul(
                        zTo_sb[:, og, RPC * ch:RPC * (ch + 1)], trp[:], INV_T)
                # allgather zhat (gpsimd ring: never blocks the x-stream)
                nc.gpsimd.dma_start(zb[ch][:], zhat[:])
                nc.gpsimd.collective_compute(
                    "AllGather", mybir.AluOpType.bypass,
                    replica_groups=[list(range(NCORES))],
                    ins=[zb[ch].opt()], outs=[zg[ch].opt()],
                )

            def emit_gather_side(ch):
                """Read back the gathered zhat and transpose into zfT."""
                zf = zfpool.tile([128, D2], f32, tag="zf")
                nc.gpsimd.dma_start(zf[:], zg[ch][:])
                for og in range(2):
                    trg = pstr.tile([120, 128], f32, tag="trg")
                    nc.tensor.transpose(trg[:], zf[:, 120 * og:120 * (og + 1)],
                                        ident_sb[:])
                    nc.vector.tensor_copy(
                        zfT_sb[:, og, 128 * ch:128 * (ch + 1)], trg[:])

            # --- software-pipelined emission ---
            for ch in range(NCHUNK):
                for i in range(SPC):
                    emit_slab(ch, i)
                    if ch == 0 and i == 2:
                        # warm the ACT tables (Sqrt, Exp) so no table loads
                        # hit the tail; placed here so the loads don't delay
                        # the first slab DMA issues on the scalar ring
                        junk = spool.tile([1, 2], f32, tag="junk")
                        nc.vector.memset(junk[:], 1.0)
                        nc.scalar.activation(junk[:, 0:1], junk[:, 1:2],
                                             AF.Square)
                        nc.scalar.sqrt(junk[:, 0:1], junk[:, 1:2])
                        nc.scalar.activation(junk[:, 0:1], junk[:, 1:2], AF.Exp)
                    # interleave previous chunk's backend inside this chunk's
                    # slab stream so no engine FIFO head-of-line blocks
                    if ch > 0 and i == 1:
                        emit_backend(ch - 1)
                    if ch > 0 and i == 4:
                        emit_gather_side(ch - 1)
            emit_backend(NCHUNK - 1)
            emit_gather_side(NCHUNK - 1)

            # --- similarity + masked exp-sum over all 512 columns ---
            s_ps = pssim.tile([BPC, B], f32, tag="sp")
            for og in range(2):
                nc.tensor.matmul(s_ps[:], zTo_sb[:, og, :], zfT_sb[:, og, :],
                                 start=(og == 0), stop=(og == 1))
            logits = spool.tile([BPC, B], f32, tag="logits")
            nc.vector.tensor_tensor(logits[:], s_ps[:], addm_sb[:], add)
            out_sb = spool.tile([BPC, 2], f32, tag="outsb")
            e_sb = spool.tile([BPC, B], f32, tag="esb")
            nc.vector.scalar_tensor_tensor(e_sb[:], logits[:], 1.0, posm_sb[:],
                                           mult, mult, accum_out=out_sb[:, 1:2])
            nc.scalar.activation(e_sb[:], logits[:], AF.Exp,
                                 bias=negsh[:], scale=1.0,
                                 accum_out=out_sb[:, 0:1])
            # keep the warmup collective alive: out += 0 * wg
            nc.vector.scalar_tensor_tensor(out_sb[:, 1:2], wg_sb[:], 0.0,
                                           out_sb[:, 1:2], mult, add)
            nc.sync.dma_start(out, out_sb[:])

    nc.compile()
    return nc


def _host_inputs(x, W1c, b1c, W2c, b2c, W1a, b1a, W2a, b2a):
    x = np.ascontiguousarray(np.asarray(x, dtype=np.float32))
    # pool window: chunk-local slab i covers chunk rows 2i, 2i+1; partition
    # p = 64*b + q (b = row within slab) must hit output column 2i + b.
    jwh = np.zeros((128, SPC, RPC), dtype=np.float16)
    p = np.arange(128)
    for i in range(SPC):
        jwh[p, i, 2 * i + p // 64] = 1.0 / S
    ident = np.eye(128, dtype=np.float32)

    def prep_mlp(W1, b1, W2, b2):
        W1 = np.asarray(W1, np.float32)
        W2 = np.asarray(W2, np.float32)
        w1h = np.zeros((128, 8, 4, 128), dtype=np.float16)
        for kc in range(8):
            k0, k1 = 128 * kc, min(128 * (kc + 1), D)
            for jg in range(4):
                w1h[0:k1 - k0, kc, jg, 0:120] = W1[k0:k1, 120 * jg:120 * (jg + 1)]
        w2h = np.zeros((120, 4, D2), dtype=np.float16)
        for kc in range(4):
            w2h[:, kc, :] = W2[120 * kc:120 * (kc + 1), :]
        b1hh = np.zeros((120, 4), dtype=np.float32)
        for jg in range(4):
            b1hh[:, jg] = np.asarray(b1, np.float32)[120 * jg:120 * (jg + 1)]
        b2rr = np.asarray(b2, np.float16).reshape(1, D2)
        return w1h, b1hh, w2h, b2rr

    mlp_c = prep_mlp(W1c, b1c, W2c, b2c)
    mlp_a = prep_mlp(W1a, b1a, W2a, b2a)

    # gathered column -> global row: col = ch*128 + c*16 + b -> c*64 + ch*16 + b
    cols = np.arange(B)
    gmap = (cols % 128) // RPC * BPC + cols // 128 * RPC + cols % RPC

    in_maps = []
    for c0 in range(NCORES):
        rows = np.arange(BPC)
        R = BPC * c0 + rows                     # global row of local row r
        addm = np.zeros((BPC, B), dtype=np.float32)
        addm[np.equal.outer(R, gmap)] = NEG_T
        posm = np.zeros((BPC, B), dtype=np.float32)
        posm[np.equal.outer((R + B // 2) % B, gmap)] = 1.0
        w1h, b1hh, w2h, b2rr = mlp_c if c0 < NCORES // 2 else mlp_a
        in_maps.append({
            "xs": x[BPC * c0:BPC * (c0 + 1)],
            "w1h": w1h, "b1h": b1hh, "w2h": w2h, "b2r": b2rr,
            "jwh": jwh, "ident": ident,
            "addm": addm, "posm": posm,
        })
    return in_maps


def kernel(x, W1c, b1c, W2c, b2c, W1a, b1a, W2a, b2a):
    global LAST_RESULT
    trace = bool(os.environ.get("BASS_TRACE"))
    if trace:
        _install_ntff_hook()
    from concourse import bass_utils
    if trace:
        bass_utils.upload_artifacts = lambda tmpdir: "local://skipped"

    if "nc" not in _CACHE:
        _CACHE["nc"] = _build_nc()
    nc = _CACHE["nc"]

    in_maps = _host_inputs(x, W1c, b1c, W2c, b2c, W1a, b1a, W2a, b2a)
    kwargs = {}
    if trace:
        kwargs = {"trace": True, "trace_cores": TRACE_CORES}
    res = bass_utils.run_bass_kernel_spmd(
        nc, in_maps, list(range(NCORES)), **kwargs)
    LAST_RESULT = res
    nll = []
    for c in range(NCORES):
        ov = res.results[c]["outv"]
        esum = ov[:, 0].astype(np.float64)
        pos = ov[:, 1].astype(np.float64)
        nll.append(SHIFT + np.log(esum) - pos)
    return np.asarray(np.concatenate(nll).mean(), dtype=np.float32)


# revision 16
# speedup vs baseline: 1.0187x; 1.0187x over previous
"""Trainium2 Bass kernel for the ESM contrastive projection head loss.

Problem (hardcoded): x [512, 512, 960] f32; two 2-layer MLPs (codon for batch
rows 0:256, amino for 256:512) applied to mean-pooled x; pairwise cosine
similarity of the concatenated projections z [512, 240]; diag-masked,
temperature-scaled InfoNCE-style NLL, mean over rows.

Strategy: data-parallel over batch across 8 NeuronCores (64 rows each).
Each core streams its 126 MB x-shard near the HBM roofline (x-slab DMAs
alternate between the two HWDGE rings), mean-pools via a DVE add-tree plus
fp16 window matmuls that accumulate pooled^T directly in PSUM.  The 64 rows
are processed in 4 pipelined chunks of 16: each chunk's MLP (fp16 weights),
row-normalize (z_hat = z/|z|), AllGather of z_hat [16,240], and post-gather
transposes all overlap the streaming of later chunks; only the last chunk's
back-end is on the critical tail.  The NxN similarity is two fp16 matmuls of
pre-normalized vectors (no norm outer product); the diagonal mask folds into
one additive mask; exp uses a fixed shift (logits <= 10) so no row max is
needed; ln(row_sum) runs on the host.  Output per core: [64, 2] = (esum, pos).
"""
import contextlib
import ctypes
import os
import sys
import types

import numpy as np

B = 512
S = 512
D = 960
NCORES = 8
BPC = B // NCORES           # 64 batch rows per core
SLAB_B = 2                  # batch rows per DMA slab
NSLAB = BPC // SLAB_B       # 32
NCHUNK = 4
SPC = NSLAB // NCHUNK       # 8 slabs per chunk
RPC = BPC // NCHUNK         # 16 rows per chunk
INV_T = 10.0                # 1 / temperature
NEG_T = -655040.0           # NEG_INF / temperature
SHIFT = 10.0                # fixed logsumexp shift (logits <= ~10)
D1 = D // 2                 # 480
D2 = D // 4                 # 240

_CACHE = {}
LAST_RESULT = None
TRACE_CORES = [0]


def _install_ntff_hook():
    """Make run_bass_kernel_spmd(trace=True) work under axon (test.py only)."""
    if "antenv.axon_hooks" in sys.modules:
        return
    so_path = "/opt/axon/libaxon_pjrt.so"
    try:
        lib = ctypes.CDLL(so_path)
    except OSError:
        return
    if not hasattr(lib, "axon_start_nrt_profile"):
        return
    lib.axon_start_nrt_profile.argtypes = [ctypes.POINTER(ctypes.c_int64), ctypes.c_size_t]
    lib.axon_start_nrt_profile.restype = ctypes.c_int64
    lib.axon_stop_nrt_profile.argtypes = [ctypes.c_char_p]
    lib.axon_stop_nrt_profile.restype = ctypes.c_int64

    @contextlib.contextmanager
    def _hook(output_dir, device_ids):
        import jax
        jax.devices()
        if device_ids:
            ids = (ctypes.c_int64 * len(device_ids))(*device_ids)
            rc = lib.axon_start_nrt_profile(ids, len(device_ids))
        else:
            rc = lib.axon_start_nrt_profile(None, 0)
        if rc != 0:
            raise RuntimeError(f"axon_start_nrt_profile rc={rc}")
        try:
            yield
        finally:
            n = lib.axon_stop_nrt_profile(str(output_dir).encode())
            print(f"profile: {n} file(s) written to {output_dir}", file=sys.stderr)

    mod = types.ModuleType("antenv.axon_hooks")
    mod.get_axon_ntff_profile_hook = lambda: _hook
    mod.set_axon_ntff_profile_hook = lambda h: None
    sys.modules["antenv.axon_hooks"] = mod


def _build_nc():
    import concourse.tile as tile
    from concourse import bacc, mybir

    f32 = mybir.dt.float32
    f16 = mybir.dt.float16
    add = mybir.AluOpType.add
    mult = mybir.AluOpType.mult
    amax = mybir.AluOpType.max
    AF = mybir.ActivationFunctionType
    AX = mybir.AxisListType

    nc = bacc.Bacc("TRN2", target_bir_lowering=False, debug=False,
                   enable_asserts=True, num_devices=NCORES)

    xs = nc.dram_tensor("xs", [BPC, S, D], f32, kind="ExternalInput").ap()
    w1h = nc.dram_tensor("w1h", [128, 8, 4, 128], f16, kind="ExternalInput").ap()
    b1h = nc.dram_tensor("b1h", [120, 4], f32, kind="ExternalInput").ap()
    w2h = nc.dram_tensor("w2h", [120, 4, D2], f16, kind="ExternalInput").ap()
    b2r = nc.dram_tensor("b2r", [1, D2], f16, kind="ExternalInput").ap()
    jwh = nc.dram_tensor("jwh", [128, SPC, RPC], f16, kind="ExternalInput").ap()
    ident = nc.dram_tensor("ident", [128, 128], f32, kind="ExternalInput").ap()
    addm = nc.dram_tensor("addm", [BPC, B], f32, kind="ExternalInput").ap()
    posm = nc.dram_tensor("posm", [BPC, B], f32, kind="ExternalInput").ap()
    out = nc.dram_tensor("outv", [BPC, 2], f32, kind="ExternalOutput").ap()

    # K-chunk sizes for the 960-feature contraction: 7x128 + 64
    KCH = [128] * 7 + [64]

    with tile.TileContext(nc) as tc:
        with contextlib.ExitStack() as ctx:
            ep = ctx.enter_context
            consts = ep(tc.tile_pool(name="consts", bufs=1))
            xpool = ep(tc.tile_pool(name="xslab", bufs=3))
            t1pool = ep(tc.tile_pool(name="t1", bufs=2))
            t2pool = ep(tc.tile_pool(name="t2", bufs=2))
            apool = ep(tc.tile_pool(name="acc", bufs=2))
            lastp = ep(tc.tile_pool(name="lastp", bufs=1))
            spool = ep(tc.tile_pool(name="small", bufs=1))
            hpool = ep(tc.tile_pool(name="hbuf", bufs=2))
            zhpool = ep(tc.tile_pool(name="zhat", bufs=2))
            zfpool = ep(tc.tile_pool(name="zfrow", bufs=2))
            dram = ep(tc.tile_pool(name="dram", bufs=1, space="DRAM"))
            ppool = ep(tc.tile_pool(name="ppool", bufs=1, space="PSUM"))
            psmm = ep(tc.tile_pool(name="psmm", bufs=1, space="PSUM"))
            psz = ep(tc.tile_pool(name="psz", bufs=1, space="PSUM"))
            pstr = ep(tc.tile_pool(name="pstr", bufs=1, space="PSUM"))
            pssim = ep(tc.tile_pool(name="pssim", bufs=1, space="PSUM"))

            # --- constants (gpsimd/SWDGE ring; x-slabs own the two HWDGE rings)
            w1_sb = consts.tile([128, 8, 4, 128], f16, tag="w1")
            nc.gpsimd.dma_start(w1_sb[:], w1h)
            w2_sb = consts.tile([120, 4, D2], f16, tag="w2")
            nc.gpsimd.dma_start(w2_sb[:], w2h)
            b1_sb = consts.tile([120, 4], f32, tag="b1")
            nc.gpsimd.dma_start(b1_sb[:], b1h)
            b2_sb = consts.tile([1, D2], f16, tag="b2")
            nc.gpsimd.dma_start(b2_sb[:], b2r)
            jw_sb = consts.tile([128, SPC, RPC], f16, tag="jw")
            nc.gpsimd.dma_start(jw_sb[:], jwh)
            ident_sb = consts.tile([128, 128], f32, tag="ident")
            nc.gpsimd.dma_start(ident_sb[:], ident)
            addm_sb = consts.tile([BPC, B], f32, tag="addm")
            nc.gpsimd.dma_start(addm_sb[:], addm)
            posm_sb = consts.tile([BPC, B], f32, tag="posm")
            nc.gpsimd.dma_start(posm_sb[:], posm)

            ones16 = consts.tile([1, RPC], f16, tag="ones16")
            nc.vector.memset(ones16[:], 1.0)
            negsh = consts.tile([BPC, 1], f32, tag="negsh")
            nc.vector.memset(negsh[:], -SHIFT)
            zeros_sb = consts.tile([120, RPC], f32, tag="zeros")
            nc.vector.memset(zeros_sb[:], 0.0)

            # --- warm up the collective path early (junk payload, folded into
            # the output so it is not dead code) ---
            wjunk = spool.tile([RPC, D2], f32, tag="wjunk")
            nc.vector.memset(wjunk[:], 0.0)
            wb = dram.tile([RPC, D2], f32, tag="wb")
            wg = dram.tile([RPC * NCORES, D2], f32, tag="wg")
            nc.gpsimd.dma_start(wb[:], wjunk[:])
            nc.gpsimd.collective_compute(
                "AllGather", mybir.AluOpType.bypass,
                replica_groups=[list(range(NCORES))],
                ins=[wb.opt()], outs=[wg.opt()],
            )
            wg_sb = spool.tile([BPC, 1], f32, tag="wg")
            nc.gpsimd.dma_start(wg_sb[:], wg[0:BPC, 0:1])

            # persistent accumulators across chunks
            zfT_sb = spool.tile([120, 2, B], f16, tag="zfT")
            zTo_sb = spool.tile([120, 2, BPC], f16, tag="zTo")

            # per-chunk DRAM collective buffers
            zb = [dram.tile([RPC, D2], f32, tag=f"zb{c}", name=f"zb{c}")
                  for c in range(NCHUNK)]
            zg = [dram.tile([RPC * NCORES, D2], f32, tag=f"zg{c}", name=f"zg{c}")
                  for c in range(NCHUNK)]

            # one PSUM bank per chunk's poolT accumulator: avoids false
            # scheduler dependencies from tile-granular tracking
            pt = [ppool.tile([128, SPC, RPC], f32, tag=f"pt{c}", name=f"pt{c}")
                  for c in range(NCHUNK)]
            hpall = psmm.tile([128, 4, RPC], f32, tag="hpall")
            back_emitted = [False] * NCHUNK

            def emit_slab(ch, i):
                """Stream slab i (2 batch rows) of chunk ch, reduce over seq,
                and accumulate pooled^T into the chunk's PSUM tile."""
                g = ch * SPC + i
                ring = nc.sync if (g % 2 == 0) else nc.scalar
                last = (ch == NCHUNK - 1 and i == SPC - 1)
                src = xs[SLAB_B * g:SLAB_B * (g + 1)].rearrange(
                    "b (q m) d -> (b q) m d", m=8)
                acc = apool.tile([128, D], f16, tag="acc")
                if not last:
                    slab = xpool.tile([128, 8, D], f32, tag="slab")
                    ring.dma_start(slab[:], src)
                    t1 = t1pool.tile([128, 4, D], f16, tag="t1")
                    nc.vector.tensor_tensor(t1[:], slab[:, 0:4, :],
                                            slab[:, 4:8, :], add)
                    t2 = t2pool.tile([128, 2, D], f16, tag="t2")
                    nc.vector.tensor_tensor(t2[:], t1[:, 0:2, :],
                                            t1[:, 2:4, :], add)
                    nc.vector.tensor_tensor(acc[:], t2[:, 0, :], t2[:, 1, :], add)
                else:
                    # final slab: 8 sub-DMAs with chained adds so the DVE tree
                    # overlaps the DMA and the serial tail is ~1 add
                    subs = [lastp.tile([128, D], f32, tag=f"sub{k}",
                                       name=f"sub{k}")
                            for k in range(8)]
                    run = lastp.tile([128, D], f32, tag="runl")
                    for k in range(8):
                        ring = nc.sync if (k % 2 == 0) else nc.scalar
                        ring.dma_start(subs[k][:], src[:, k, :])
                        if k == 1:
                            nc.vector.tensor_tensor(run[:], subs[0][:],
                                                    subs[1][:], add)
                        elif k >= 2:
                            dst = acc if k == 7 else run
                            nc.vector.tensor_tensor(dst[:], run[:],
                                                    subs[k][:], add)
                for g8 in range(8):
                    K = KCH[g8]
                    # one accumulation group per chunk: start=True clears the
                    # has-written bits of the WHOLE bank, so only the first
                    # matmul of the chunk may carry it
                    nc.tensor.matmul(
                        pt[ch][0:K, g8, :],
                        acc[:, 128 * g8:128 * g8 + K],
                        jw_sb[:, i, :],
                        start=(i == 0 and g8 == 0),
                        stop=(i == SPC - 1 and g8 == 7),
                        skip_group_check=True)

            def emit_backend(ch):
                """MLP + normalize + payload DMA + AllGather for chunk ch."""
                back_emitted[ch] = True
                pT = spool.tile([128, SPC, RPC], f16, tag=f"pT{ch}")
                nc.vector.tensor_copy(pT[:], pt[ch][:])
                # MLP layer 1: h [120, 4, RPC] = relu(W1^T pooled^T + b1)
                h_sb = hpool.tile([120, 4, RPC], f16, tag="h")
                for jg in range(4):
                    hp = hpall[:, jg]
                    for kc in range(8):
                        K = KCH[kc]
                        nc.tensor.matmul(hp[:], w1_sb[0:K, kc, jg, :],
                                         pT[0:K, kc, :],
                                         start=(kc == 0), stop=(kc == 7))
                    nc.vector.scalar_tensor_tensor(
                        h_sb[:, jg, :], hp[0:120, :], b1_sb[:, jg:jg + 1],
                        zeros_sb[:], add, amax)
                # MLP layer 2 (row-major): z [RPC, 240] = h^T W2 + b2
                zp = psz.tile([RPC, D2], f32, tag="zp")
                for kc in range(4):
                    nc.tensor.matmul(zp[:], h_sb[:, kc, :], w2_sb[:, kc, :],
                                     start=(kc == 0), stop=False)
                nc.tensor.matmul(zp[:], ones16[:], b2_sb[:],
                                 start=False, stop=True)
                # normalize rows: zhat = z / |z|  (Square+accum gives the
                # row sum-of-squares in one ACT op)
                sq = zhpool.tile([RPC, D2], f32, tag="sq")
                nsq = spool.tile([RPC, 1], f32, tag=f"nsq{ch}")
                nc.scalar.activation(sq[:], zp[:], AF.Square,
                                     accum_out=nsq[:])
                nst = spool.tile([RPC, 1], f32, tag=f"nst{ch}")
                nc.scalar.sqrt(nst[:], nsq[:])
                nlr = spool.tile([RPC, 1], f32, tag=f"nlr{ch}")
                nc.vector.reciprocal(nlr[:], nst[:])
                zhat = zhpool.tile([RPC, D2], f32, tag="zhat")
                nc.vector.tensor_scalar_mul(zhat[:], zp[:], nlr[:])
                # own z^T blocks (scaled by 1/T) via PE transpose of zhat
                for og in range(2):
                    trp = pstr.tile([120, 128], f32, tag="tr", name="trp")
                    nc.tensor.transpose(trp[:, 0:RPC],
                                        zhat[:, 120 * og:120 * (og + 1)],
                                        ident_sb[0:RPC, 0:RPC])
                    nc.vector.tensor_scalar_mul(
                        zTo_sb[:, og, RPC * ch:RPC * (ch + 1)], trp[:, 0:RPC],
                        INV_T)
                # allgather zhat (gpsimd ring: never blocks the x-stream)
                nc.gpsimd.dma_start(zb[ch][:], zhat[:])
                nc.gpsimd.collective_compute(
                    "AllGather", mybir.AluOpType.bypass,
                    replica_groups=[list(range(NCORES))],
                    ins=[zb[ch].opt()], outs=[zg[ch].opt()],
                )

            def emit_gather_side(ch):
                """Read back the gathered zhat and transpose into zfT."""
                zf = zfpool.tile([128, D2], f32, tag="zf")
                nc.gpsimd.dma_start(zf[:], zg[ch][:])
                for og in range(2):
                    trg = pstr.tile([120, 128], f32, tag="tr", name="trg")
                    nc.tensor.transpose(trg[:], zf[:, 120 * og:120 * (og + 1)],
                                        ident_sb[:])
                    nc.vector.tensor_copy(
                        zfT_sb[:, og, 128 * ch:128 * (ch + 1)], trg[:])

            # --- software-pipelined emission ---
            for ch in range(NCHUNK):
                for i in range(SPC):
                    emit_slab(ch, i)
                    if ch == 0 and i == 2:
                        # warm the ACT tables (Sqrt, Exp) so no table loads
                        # hit the tail; placed here so the loads don't delay
                        # the first slab DMA issues on the scalar ring
                        junk = spool.tile([1, 2], f32, tag="junk")
                        nc.vector.memset(junk[:], 1.0)
                        nc.scalar.activation(junk[:, 0:1], junk[:, 1:2],
                                             AF.Square)
                        nc.scalar.sqrt(junk[:, 0:1], junk[:, 1:2])
                        nc.scalar.activation(junk[:, 0:1], junk[:, 1:2], AF.Exp)
                    # interleave previous chunk's backend inside this chunk's
                    # slab stream so no engine FIFO head-of-line blocks
                    if ch > 0 and i == 1:
                        with tc.high_priority():
                            emit_backend(ch - 1)
                    if ch > 0 and i == 4:
                        with tc.high_priority():
                            emit_gather_side(ch - 1)
            emit_backend(NCHUNK - 1)
            emit_gather_side(NCHUNK - 1)

            # --- similarity + masked exp-sum over all 512 columns ---
            s_ps = pssim.tile([BPC, B], f32, tag="sp")
            for og in range(2):
                nc.tensor.matmul(s_ps[:], zTo_sb[:, og, :], zfT_sb[:, og, :],
                                 start=(og == 0), stop=(og == 1))
            logits = spool.tile([BPC, B], f32, tag="logits")
            nc.vector.tensor_tensor(logits[:], s_ps[:], addm_sb[:], add)
            out_sb = spool.tile([BPC, 2], f32, tag="outsb")
            e_sb = spool.tile([BPC, B], f32, tag="esb")
            nc.vector.scalar_tensor_tensor(e_sb[:], logits[:], 1.0, posm_sb[:],
                                           mult, mult, accum_out=out_sb[:, 1:2])
            nc.scalar.activation(e_sb[:], logits[:], AF.Exp,
                                 bias=negsh[:], scale=1.0,
                                 accum_out=out_sb[:, 0:1])
            # keep the warmup collective alive: out += 0 * wg
            nc.vector.scalar_tensor_tensor(out_sb[:, 1:2], wg_sb[:], 0.0,
                                           out_sb[:, 1:2], mult, add)
            nc.sync.dma_start(out, out_sb[:])

    nc.compile()
    return nc


def _host_inputs(x, W1c, b1c, W2c, b2c, W1a, b1a, W2a, b2a):
    x = np.ascontiguousarray(np.asarray(x, dtype=np.float32))
    # pool window: chunk-local slab i covers chunk rows 2i, 2i+1; partition
    # p = 64*b + q (b = row within slab) must hit output column 2i + b.
    jwh = np.zeros((128, SPC, RPC), dtype=np.float16)
    p = np.arange(128)
    for i in range(SPC):
        jwh[p, i, 2 * i + p // 64] = 1.0 / S
    ident = np.eye(128, dtype=np.float32)

    def prep_mlp(W1, b1, W2, b2):
        W1 = np.asarray(W1, np.float32)
        W2 = np.asarray(W2, np.float32)
        w1h = np.zeros((128, 8, 4, 128), dtype=np.float16)
        for kc in range(8):
            k0, k1 = 128 * kc, min(128 * (kc + 1), D)
            for jg in range(4):
                w1h[0:k1 - k0, kc, jg, 0:120] = W1[k0:k1, 120 * jg:120 * (jg + 1)]
        w2h = np.zeros((120, 4, D2), dtype=np.float16)
        for kc in range(4):
            w2h[:, kc, :] = W2[120 * kc:120 * (kc + 1), :]
        b1hh = np.zeros((120, 4), dtype=np.float32)
        for jg in range(4):
            b1hh[:, jg] = np.asarray(b1, np.float32)[120 * jg:120 * (jg + 1)]
        b2rr = np.asarray(b2, np.float16).reshape(1, D2)
        return w1h, b1hh, w2h, b2rr

    mlp_c = prep_mlp(W1c, b1c, W2c, b2c)
    mlp_a = prep_mlp(W1a, b1a, W2a, b2a)

    # gathered column -> global row: col = ch*128 + c*16 + b -> c*64 + ch*16 + b
    cols = np.arange(B)
    gmap = (cols % 128) // RPC * BPC + cols // 128 * RPC + cols % RPC

    in_maps = []
    for c0 in range(NCORES):
        rows = np.arange(BPC)
        R = BPC * c0 + rows                     # global row of local row r
        addm = np.zeros((BPC, B), dtype=np.float32)
        addm[np.equal.outer(R, gmap)] = NEG_T
        posm = np.zeros((BPC, B), dtype=np.float32)
        posm[np.equal.outer((R + B // 2) % B, gmap)] = 1.0
        w1h, b1hh, w2h, b2rr = mlp_c if c0 < NCORES // 2 else mlp_a
        in_maps.append({
            "xs": x[BPC * c0:BPC * (c0 + 1)],
            "w1h": w1h, "b1h": b1hh, "w2h": w2h, "b2r": b2rr,
            "jwh": jwh, "ident": ident,
            "addm": addm, "posm": posm,
        })
    return in_maps


def kernel(x, W1c, b1c, W2c, b2c, W1a, b1a, W2a, b2a):
    global LAST_RESULT
    trace = bool(os.environ.get("BASS_TRACE"))
    if trace:
        _install_ntff_hook()
    from concourse import bass_utils
    if trace:
        bass_utils.upload_artifacts = lambda tmpdir: "local://skipped"

    if "nc" not in _CACHE:
        _CACHE["nc"] = _build_nc()
    nc = _CACHE["nc"]

    in_maps = _host_inputs(x, W1c, b1c, W2c, b2c, W1a, b1a, W2a, b2a)
    kwargs = {}
    if trace:
        kwargs = {"trace": True, "trace_cores": TRACE_CORES}
    res = bass_utils.run_bass_kernel_spmd(
        nc, in_maps, list(range(NCORES)), **kwargs)
    LAST_RESULT = res
    nll = []
    for c in range(NCORES):
        ov = res.results[c]["outv"]
        esum = ov[:, 0].astype(np.float64)
        pos = ov[:, 1].astype(np.float64)
        nll.append(SHIFT + np.log(esum) - pos)
    return np.asarray(np.concatenate(nll).mean(), dtype=np.float32)
